# revision 18
# baseline (speedup 1.0000x reference)
"""QLoRA-style MLP (NF4 dequant + LoRA + SiLU) on 8 TRN2 NeuronCores.

Sharding: 4-way d_ff (tensor parallel) x 2-way tokens (data parallel).
Each core dequantizes its d_ff shard of both weight matrices on-device
(deg-9 endpoint-constrained polynomial of the NF4 codebook evaluated as
a fused scalar_tensor_tensor Horner chain on DVE, 11 ops/element), runs
fp16 matmuls with fp32 PSUM accumulation, fuses the LoRA terms into the
same PSUM groups, applies SiLU on the ACT engine, and reduces the
partial down-projection outputs across the 4 cores of each token half
with fine-grained (per panel x token-block) ReduceScatters.

Host-side work is limited to lossless layout transforms: transpose /
slice / pad / value-preserving dtype casts.
"""

import math

import numpy as np

import concourse.bass as bass
import concourse.bacc as bacc
import concourse.mybir as mybir
from concourse import bass_utils
from concourse.tile import TileContext

F16 = mybir.dt.float16
F32 = mybir.dt.float32
AF = mybir.ActivationFunctionType
ALU = mybir.AluOpType

# NF4 codebook (bitsandbytes dequantize_4bit)
NF4 = np.array([
    -1.0, -0.6961928009986877, -0.5250730514526367, -0.39491748809814453,
    -0.28444138169288635, -0.18477343022823334, -0.09105003625154495, 0.0,
    0.07958029955625534, 0.16093020141124725, 0.24611230194568634,
    0.33791524171829224, 0.44070982933044434, 0.5626170039176941,
    0.7229568362236023, 1.0], dtype=np.float64)


def _fit_poly9():
    """Endpoint-constrained deg-9 least-squares fit of NF4 over
    z=(c-7.5)/7.5.  Exact at codes 0 and 15; rms 9e-4 over the table when
    evaluated in fp16.  Returns coefficients low->high."""
    c = np.arange(16.0)
    z = (c - 7.5) / 7.5
    deg = 9
    V = np.vander(z, deg + 1, increasing=True)
    A = np.vstack([np.vander(np.array([-1.0]), deg + 1, increasing=True),
                   np.vander(np.array([1.0]), deg + 1, increasing=True)])
    K = np.block([[2 * V.T @ V, A.T], [A, np.zeros((2, 2))]])
    rhs = np.concatenate([2 * V.T @ NF4, np.array([-1.0, 1.0])])
    cf = np.linalg.solve(K, rhs)[:deg + 1]
    return [float(x) for x in cf]


COEF = _fit_poly9()

# ---------------------------------------------------------------- config

FULL_CFG = dict(
    D=4096,          # d_model
    FSH=2816,        # d_ff shard per core (d_ff padded 11008 -> 11264 = 4*2816)
    TSH=4096,        # tokens per core (8192 = 2*4096)
    TW=512,          # token tile width (one PSUM bank)
    FP=8,            # f-tiles dequantized per up panel
    MP=8,            # m-tiles dequantized per down panel
    CHT_UP=8,        # dequant chunk, in 128-col tiles, up
    CHT_DN=11,       # dequant chunk, in 128-col tiles, down
    R=16,            # lora rank
    N_CORES=8,
    GROUPS=[[0, 1, 2, 3], [4, 5, 6, 7]],
)


def _emit_strip(nc, wp, strip, codes_dram, scales16, col_off, nt, cht,
                uid, ctag):
    """Dequantize one weight strip.

    strip: SBUF tile [128, nt, 128] fp16 (partition = contraction index
    within tile, free = (tile index, output col within tile)).
    codes_dram: DRAM AP [128, nt, 128] fp16 codes for this strip.
    scales16: DRAM tile [2*nt_total, n_cols] fp16; scale for partition p,
    tile k, col w is scales16[2*k + p//64, col_off + w].

    w = poly9((c-7.5)/7.5) * scale via a fused Horner chain:
      z = (c - 7.5) * (1/7.5)                    TENSOR_SCALAR
      v = z * a9                                 TENSOR_SCALAR
      v = (v + a_k) * z   for k = 8..1           SCALAR_TENSOR_TENSOR x8
      w = (v + a0) * scale                       SCALAR_TENSOR_TENSOR
    """
    ch = cht * 128
    fcht = min(2 * cht, nt)          # codes/scale feed granularity
    fch = fcht * 128
    sflat = strip.rearrange("p a b -> p (a b)")
    scr = scales16.rearrange("(k two) f -> two k f", two=2)
    for f0 in range(0, nt, fcht):
        # coarse codes load + scale broadcast (a fine-grained feed was the
        # dequant bottleneck: chunk cadence was DMA-latency-, not DVE-, bound)
        ftag = f"{ctag}{uid}_{f0}"
        cch = wp.tile([128, fch], F16, tag=f"{ctag}cd", bufs=2,
                      name=f"cd{ftag}")
        nc.scalar.dma_start(
            cch.rearrange("p (a b) -> p a b", b=128),
            codes_dram[:, f0:f0 + fcht, :])
        scl = wp.tile([128, fch], F16, tag=f"{ctag}sc", bufs=2,
                      name=f"sc{ftag}")
        scl3 = scl.rearrange("p (a b) -> p a b", b=128)
        for hi in (0, 1):
            bsrc = scr[hi, f0:f0 + fcht, col_off:col_off + 128]
            bsrc = bsrc.unsqueeze(0).to_broadcast([64, fcht, 128])
            nc.gpsimd.dma_start(scl3[64 * hi:64 * (hi + 1), :, :], bsrc)
        for k0 in range(f0, f0 + fcht, cht):
            tag = f"{ctag}{uid}_{k0}"
            sl = slice(k0 * 128, (k0 + cht) * 128)
            fsl = slice((k0 - f0) * 128, (k0 - f0 + cht) * 128)
            z = wp.tile([128, ch], F16, tag=f"{ctag}z", bufs=2,
                        name=f"z{tag}")
            nc.vector.tensor_scalar(z[:], cch[:, fsl], -7.5, 1.0 / 7.5,
                                    ALU.add, ALU.mult)
            v = wp.tile([128, ch], F16, tag=f"{ctag}v", bufs=3,
                        name=f"v9{tag}")
            nc.vector.tensor_scalar(v[:], z[:], COEF[9], 0.0, ALU.mult,
                                    ALU.add)
            for k in range(8, 0, -1):
                v2 = wp.tile([128, ch], F16, tag=f"{ctag}v", bufs=3,
                             name=f"v{k}{tag}")
                nc.vector.scalar_tensor_tensor(v2[:], v[:], COEF[k], z[:],
                                               ALU.add, ALU.mult)
                v = v2
            nc.vector.scalar_tensor_tensor(sflat[:, sl], v[:], COEF[0],
                                           scl[:, fsl], ALU.add, ALU.mult)


def build_nc(cfg):
    D, FSH, TSH, TW = cfg["D"], cfg["FSH"], cfg["TSH"], cfg["TW"]
    FP, MP, R = cfg["FP"], cfg["MP"], cfg["R"]
    CHT_UP, CHT_DN = cfg["CHT_UP"], cfg["CHT_DN"]
    KT, FT, MT, TB = D // 128, FSH // 128, D // 128, TSH // TW
    G = len(cfg["GROUPS"][0])
    N_RS = MT // MP              # ReduceScatter row-panels
    MPR = (MP * 128) // G        # output rows per RS panel

    nc = bacc.Bacc(None, num_devices=cfg["N_CORES"], num_swdge_queues=4)

    xT = nc.dram_tensor("xT", [D, TSH], F32, kind="ExternalInput")
    up_codes = nc.dram_tensor("up_codes", [128, FT, KT, 128], F16,
                              kind="ExternalInput")
    up_scales = nc.dram_tensor("up_scales", [D // 64, FSH], F32,
                               kind="ExternalInput")
    dn_codes = nc.dram_tensor("dn_codes", [128, MT, FT, 128], F16,
                              kind="ExternalInput")
    dn_scales = nc.dram_tensor("dn_scales", [FSH // 64, D], F32,
                               kind="ExternalInput")
    up_a = nc.dram_tensor("up_a", [128, KT, R], F32, kind="ExternalInput")
    up_b = nc.dram_tensor("up_b", [R, FSH], F32, kind="ExternalInput")
    dn_a = nc.dram_tensor("dn_a", [128, FT, R], F32, kind="ExternalInput")
    dn_b = nc.dram_tensor("dn_b", [R, D], F32, kind="ExternalInput")
    y2c = nc.dram_tensor("y2c", [N_RS, TB, MPR, TW], F16,
                         kind="ExternalOutput")

    with TileContext(nc) as tc:
        with tc.tile_pool(name="dram", bufs=1, space="DRAM") as dram:
            x2T16 = dram.tile([128, FT, TSH], F16)
            upsc16 = dram.tile([D // 64, FSH], F16)
            dnsc16 = dram.tile([FSH // 64, D], F16)
            # per-(panel, token-block) contiguous slabs for the collectives
            y2p = dram.tile([N_RS, TB, MP * 128, TW], F16)
            rs16 = dram.tile([N_RS, TB, MPR, TW], F16)

            xv = xT.rearrange("(kt ki) t -> ki kt t", ki=128)

            # ------------------------------------------------ prep phase
            with tc.tile_pool(name="consts", bufs=1) as cp:
                up_a16 = cp.tile([128, KT, R], F16)
                up_b16 = cp.tile([R, FSH], F16)
                tT = cp.tile([R, TSH], F16)
                with tc.tile_pool(name="prep", bufs=2) as pp:
                    s32 = pp.tile([D // 64, FSH], F32, tag="sc32", name="us32")
                    nc.sync.dma_start(s32[:], up_scales[:])
                    s16 = pp.tile([D // 64, FSH], F16, tag="sc16", name="us16")
                    nc.vector.tensor_copy(s16[:], s32[:])
                    nc.sync.dma_start(upsc16[:], s16[:])
                    d32 = pp.tile([FSH // 64, D], F32, tag="sc32", name="ds32")
                    nc.sync.dma_start(d32[:], dn_scales[:])
                    d16 = pp.tile([FSH // 64, D], F16, tag="sc16", name="ds16")
                    nc.vector.tensor_copy(d16[:], d32[:])
                    nc.sync.dma_start(dnsc16[:], d16[:])
                    a32 = pp.tile([128, KT, R], F32, tag="lora", name="ua32")
                    nc.sync.dma_start(a32[:], up_a[:])
                    nc.vector.tensor_copy(up_a16[:], a32[:])
                    b32 = pp.tile([R, FSH], F32, tag="lorab", name="ub32")
                    nc.sync.dma_start(b32[:], up_b[:])
                    nc.vector.tensor_copy(up_b16[:], b32[:])

                # ---------------------------------------------- up phase
                n_panels = math.ceil(FT / FP)
                with (
                    tc.tile_pool(name="ustrip", bufs=FP + 4) as sp,
                    tc.tile_pool(name="uwork", bufs=2) as wp,
                    tc.tile_pool(name="ux", bufs=2) as xp,
                    tc.tile_pool(name="ups", bufs=2, space="PSUM") as psp,
                    tc.tile_pool(name="ustage", bufs=3) as stg,
                ):
                    for p in range(n_panels):
                        fts = list(range(p * FP, min((p + 1) * FP, FT)))
                        strips = {}
                        for f in fts:
                            strip = sp.tile([128, KT, 128], F16, tag="ustrip",
                                            bufs=FP + 4, name=f"ustrip{f}")
                            _emit_strip(nc, wp, strip, up_codes[:, f, :, :],
                                        upsc16, 128 * f, KT, CHT_UP, f, "u")
                            strips[f] = strip
                        for t in range(TB):
                            # x tile: fp32 DRAM -> fp16 SBUF casting DMAs,
                            # split in two half-k tiles so the next block's
                            # loads start as soon as the first half frees
                            KH = KT // 2
                            xth = []
                            for h in range(2):
                                xh = xp.tile([128, KH, TW], F16,
                                             tag=f"xt{h}", bufs=2,
                                             name=f"xt{p}_{t}_{h}")
                                for kk in range(KH):
                                    nc.gpsimd.dma_start(
                                        xh[:, kk, :],
                                        xv[:, h * KH + kk,
                                           TW * t:TW * (t + 1)])
                                xth.append(xh)

                            def xt_sl(kt):
                                return xth[kt // KH][:, kt % KH, :]
                            if p == 0:
                                pt = psp.tile([R, TW], F32, tag="ptT", bufs=2,
                                              name=f"ptT{t}")
                                for kt in range(KT):
                                    nc.tensor.matmul(pt[:], up_a16[:, kt, :],
                                                     xt_sl(kt),
                                                     start=(kt == 0),
                                                     stop=(kt == KT - 1))
                                nc.scalar.copy(
                                    tT[:, TW * t:TW * (t + 1)], pt[:])
                            for f in fts:
                                ps = psp.tile([128, TW], F32, tag="py1",
                                              bufs=6, name=f"py1_{f}_{t}")
                                for kt in range(KT):
                                    nc.tensor.matmul(ps[:],
                                                     strips[f][:, kt, :],
                                                     xt_sl(kt),
                                                     start=(kt == 0),
                                                     stop=False)
                                nc.tensor.matmul(
                                    ps[:], up_b16[:, 128 * f:128 * (f + 1)],
                                    tT[:, TW * t:TW * (t + 1)],
                                    start=False, stop=True)
                                so = stg.tile([128, TW], F16, tag="silu",
                                              bufs=3, name=f"so{f}_{t}")
                                nc.scalar.activation(so[:], ps[:], AF.Silu)
                                nc.scalar.dma_start(
                                    x2T16[:, f, TW * t:TW * (t + 1)], so[:])

                # -------------------------------------------- down phase
                n_dpanels = MT // MP
                pending_rs = []
                pending_y2c = []
                with (
                    tc.tile_pool(name="dconsts", bufs=1) as dcp,
                    tc.tile_pool(name="dstrip", bufs=MP + 5) as dsp,
                    tc.tile_pool(name="dwork", bufs=2) as dwp,
                    tc.tile_pool(name="dx", bufs=2) as dxp,
                    tc.tile_pool(name="dps", bufs=2, space="PSUM") as dpsp,
                    tc.tile_pool(name="dstage", bufs=3) as dstg,
                ):
                    dn_a16 = dcp.tile([128, FT, R], F16)
                    dn_b16 = dcp.tile([R, D], F16)
                    t2T = dcp.tile([R, TSH], F16)
                    with tc.tile_pool(name="dprep", bufs=1) as dpp:
                        da32 = dpp.tile([128, FT, R], F32, name="da32")
                        nc.sync.dma_start(da32[:], dn_a[:])
                        nc.vector.tensor_copy(dn_a16[:], da32[:])
                        db32 = dpp.tile([R, D], F32, name="db32")
                        nc.sync.dma_start(db32[:], dn_b[:])
                        nc.vector.tensor_copy(dn_b16[:], db32[:])
                    for dp in range(n_dpanels):
                        mts = list(range(dp * MP, (dp + 1) * MP))
                        dstrips = {}
                        for m in mts:
                            strip = dsp.tile([128, FT, 128], F16, tag="dstrip",
                                             bufs=MP + 5, name=f"dstrip{m}")
                            _emit_strip(nc, dwp, strip, dn_codes[:, m, :, :],
                                        dnsc16, 128 * m, FT, CHT_DN, m, "d")
                            dstrips[m] = strip
                        for t in range(TB):
                            x2t = dxp.tile([128, FT, TW], F16, tag="x2t",
                                           bufs=2, name=f"x2t{dp}_{t}")
                            nc.sync.dma_start(
                                x2t[:], x2T16[:, :, TW * t:TW * (t + 1)])
                            if dp == 0:
                                pt2 = dpsp.tile([R, TW], F32, tag="pt2",
                                                bufs=2, name=f"pt2_{t}")
                                for ft in range(FT):
                                    nc.tensor.matmul(pt2[:], dn_a16[:, ft, :],
                                                     x2t[:, ft, :],
                                                     start=(ft == 0),
                                                     stop=(ft == FT - 1))
                                nc.scalar.copy(
                                    t2T[:, TW * t:TW * (t + 1)], pt2[:])
                            for m in mts:
                                ps = dpsp.tile([128, TW], F32, tag="py2",
                                               bufs=6, name=f"py2_{m}_{t}")
                                for ft in range(FT):
                                    nc.tensor.matmul(ps[:],
                                                     dstrips[m][:, ft, :],
                                                     x2t[:, ft, :],
                                                     start=(ft == 0),
                                                     stop=False)
                                nc.tensor.matmul(
                                    ps[:], dn_b16[:, 128 * m:128 * (m + 1)],
                                    t2T[:, TW * t:TW * (t + 1)],
                                    start=False, stop=True)
                                po = dstg.tile([128, TW], F16, tag="pout",
                                               bufs=3, name=f"po{m}_{t}")
                                nc.scalar.copy(po[:], ps[:])
                                ml = m - dp * MP
                                nc.scalar.dma_start(
                                    y2p[dp, t, 128 * ml:128 * (ml + 1), :],
                                    po[:])
                            # reduce (panel, token-block) across the 4
                            # cores of the token half; dispatch is deferred
                            # 2 blocks so the queue head never waits on
                            # in-flight slab writes
                            pending_rs.append((dp, t))
                            if len(pending_rs) > 2:
                                rdp, rt_ = pending_rs.pop(0)
                                nc.gpsimd.collective_compute(
                                    "ReduceScatter",
                                    ALU.add,
                                    replica_groups=cfg["GROUPS"],
                                    ins=[y2p[rdp, rt_, :, :].opt()],
                                    outs=[rs16[rdp, rt_, :, :].opt()],
                                )
                                pending_y2c.append((rdp, rt_))
                            if len(pending_y2c) > 2:
                                pdp, pt_ = pending_y2c.pop(0)
                                nc.sync.dma_start(y2c[pdp, pt_, :, :],
                                                  rs16[pdp, pt_, :, :])
                    for rdp, rt_ in pending_rs:
                        nc.gpsimd.collective_compute(
                            "ReduceScatter",
                            ALU.add,
                            replica_groups=cfg["GROUPS"],
                            ins=[y2p[rdp, rt_, :, :].opt()],
                            outs=[rs16[rdp, rt_, :, :].opt()],
                        )
                        pending_y2c.append((rdp, rt_))
                    for pdp, pt_ in pending_y2c:
                        nc.sync.dma_start(y2c[pdp, pt_, :, :],
                                          rs16[pdp, pt_, :, :])
    nc.compile()
    return nc


# ---------------------------------------------------------------- host side

def _tile_codes_k_major(codesT):
    """codesT [K, F] -> [128, F//128, K//128, 128] fp16 (ki, ft, kt, fw)."""
    K, F = codesT.shape
    a = codesT.reshape(K // 128, 128, F // 128, 128)
    return np.ascontiguousarray(a.transpose(1, 2, 0, 3)).astype(np.float16)


def prep_inputs(inputs, cfg):
    D, FSH, TSH, R = cfg["D"], cfg["FSH"], cfg["TSH"], cfg["R"]
    n_cores = cfg["N_CORES"]
    n_ff = len(cfg["GROUPS"][0])
    DFF = inputs["w_up_codes"].shape[0]
    FFP = FSH * n_ff

    x1 = np.asarray(inputs["x1"], np.float32)
    xT_full = np.ascontiguousarray(x1.T)                     # [D, N_TOK]

    upc = np.full((FFP, D), 7, np.int32)
    upc[:DFF] = inputs["w_up_codes"]
    upam = np.ones((FFP, D // 64), np.float32)
    upam[:DFF] = np.asarray(inputs["w_up_absmax"],
                            np.float32).reshape(DFF, D // 64)
    dnc = np.full((D, FFP), 7, np.int32)
    dnc[:, :DFF] = inputs["w_down_codes"]
    dnam = np.ones((D, FFP // 64), np.float32)
    dnam[:, :DFF // 64] = np.asarray(
        inputs["w_down_absmax"], np.float32).reshape(D, DFF // 64)
    upb = np.zeros((R, FFP), np.float32)
    upb[:, :DFF] = inputs["w_up_lora_b"]
    dna = np.zeros((FFP, R), np.float32)
    dna[:DFF] = inputs["w_down_lora_a"]

    up_a_t = np.ascontiguousarray(
        np.asarray(inputs["w_up_lora_a"], np.float32)
        .reshape(D // 128, 128, R).transpose(1, 0, 2))
    dn_b_full = np.ascontiguousarray(
        np.asarray(inputs["w_down_lora_b"], np.float32))

    in_maps = []
    for c in range(n_cores):
        q, hh = c % n_ff, c // n_ff
        fsl = slice(q * FSH, (q + 1) * FSH)
        bsl = slice(q * (FSH // 64), (q + 1) * (FSH // 64))
        tsl = slice(hh * TSH, (hh + 1) * TSH)
        up_codesT = np.ascontiguousarray(upc[fsl].T)          # [D, FSH]
        dn_codesT = np.ascontiguousarray(dnc[:, fsl].T)       # [FSH, D]
        in_maps.append(dict(
            xT=np.ascontiguousarray(xT_full[:, tsl]),
            up_codes=_tile_codes_k_major(up_codesT),
            up_scales=np.ascontiguousarray(upam[fsl].T),      # [D//64, FSH]
            dn_codes=_tile_codes_k_major(dn_codesT),
            dn_scales=np.ascontiguousarray(dnam[:, bsl].T),   # [FSH//64, D]
            up_a=up_a_t,
            up_b=np.ascontiguousarray(upb[:, fsl]),
            dn_a=np.ascontiguousarray(
                dna[fsl].reshape(FSH // 128, 128, R).transpose(1, 0, 2)),
            dn_b=dn_b_full,
        ))
    return in_maps


def assemble(outs, cfg):
    D, TSH, MP = cfg["D"], cfg["TSH"], cfg["MP"]
    n_cores = cfg["N_CORES"]
    n_ff = len(cfg["GROUPS"][0])
    n_t = n_cores // n_ff
    MT = D // 128
    N_RS = MT // MP
    MPR = (MP * 128) // n_ff
    N_TOK = TSH * n_t
    TW = cfg["TW"]
    TB = TSH // TW
    y2T = np.zeros((D, N_TOK), np.float32)
    for c in range(n_cores):
        q, hh = c % n_ff, c // n_ff
        out = np.asarray(outs[c], np.float32)      # [N_RS, TB, MPR, TW]
        for dp in range(N_RS):
            gm = MP * 128 * dp + MPR * q
            y2T[gm:gm + MPR, TSH * hh:TSH * (hh + 1)] = \
                out[dp].transpose(1, 0, 2).reshape(MPR, TSH)
    return np.ascontiguousarray(y2T.T)


_NC_CACHE = {}


def kernel(**inputs):
    cfg = FULL_CFG
    if "full" not in _NC_CACHE:
        _NC_CACHE["full"] = build_nc(cfg)
    nc = _NC_CACHE["full"]
    in_maps = prep_inputs(inputs, cfg)
    res = bass_utils.run_bass_kernel_spmd(
        nc, in_maps, core_ids=list(range(cfg["N_CORES"])))
    return assemble([r["y2c"] for r in res.results], cfg)
